# revision 1
# baseline (speedup 1.0000x reference)
"""Trainium2 Bass kernel for the neural-backflow problem.

Problem (hardcoded shapes): rs (4096, 3) f32 in a periodic box L=10.
For every electron pair (i, j): minimum-image displacement d_ij, distance
r_ij, force f_ij = MLP_spin(r_ij) (1->32->1 swish MLP with compact-support
decay; "same" weights for same-spin pairs, "diff" for cross-spin), output
rs + sum_j f_ij * d_ij.

Key algebraic reduction used here: with z_k = decay*w1_k + b1_k,
  force = decay^2 * sum_k (w1_k*wo_k) * sigmoid(z_k) + bo*decay
which is a smooth scalar function P(decay) on decay in (0, 1].  We fit a
degree-10 polynomial (Chebyshev fit, monomial coeffs, P(0)=0 forced) to P at
kernel-call time from the actual weight values, so the device program is
input-independent: the MLP collapses to a Horner chain of
scalar_tensor_tensor ops on the Vector engine.

decay itself is computed exactly (not approximated):
  m   = ((rs_j - rs_i + 15) mod 10) - 5          (= -minimum-image disp)
  r2  = m_x^2 + m_y^2 + m_z^2                     (matches sqrt(r2+1e-15)^2)
  g   = clamp(1 - 0.04*(r2 + 1e-15), >= 1-(1-1e-5)^2)   (= 1 - xn^2)
  decay = exp(1 - 1/g),  with 1/g = exp(-ln g) on the ACT engine
  (ScalarE Reciprocal is banned; Ln/Exp/Square/Copy share one ACT table set)

Sharding: rows of the pair grid across 8 cores (512 rows each); rs is
replicated (pre-broadcast across 128 partitions host-side for the j-axis
tiles).  Row-sums are local per core; outputs are concatenated.
"""

import numpy as np

import concourse.bass as bass
import concourse.mybir as mybir
from concourse.tile import TileContext
from concourse.bass_utils import run_bass_kernel_spmd

L = 10.0
N = 4096
N_UP = 2048
NCORES = 8
ROWS = N // NCORES          # 512 rows per core
JT = 512                    # j-tile width
NJT = N // JT               # 8 j-tiles
NIB = ROWS // 128           # 4 i-blocks of 128 rows per core
DEG = 10                    # polynomial degree
GMIN = float(np.float32(1.0) - np.float32((1.0 - 1e-5) ** 2))

F32 = mybir.dt.float32
AOP = mybir.AluOpType
AF = mybir.ActivationFunctionType

LAST_RESULTS = None  # BassKernelResults of the most recent run (for profiling)
_CACHED = {}         # built Bass program, keyed by nothing (shapes are fixed)


def _fit_poly(w1, b1, wo, bo):
    """Degree-DEG monomial coeffs of P(d) = d^2*S(d) + bo*d on d in [0,1],
    S(d) = sum_k w1_k*wo_k*sigmoid(w1_k*d + b1_k).  Returns c[1..DEG]
    (c[0] is forced to 0 exactly)."""
    w1 = np.asarray(w1, np.float64).ravel()
    b1 = np.asarray(b1, np.float64).ravel()
    wo = np.asarray(wo, np.float64).ravel()
    bo = float(np.asarray(bo, np.float64).ravel()[0])
    c = w1 * wo
    d = np.linspace(0.0, 1.0, 20001)
    z = d[:, None] * w1[None, :] + b1[None, :]
    S = (c[None, :] / (1.0 + np.exp(-z))).sum(axis=1)
    P = d * d * S + bo * d
    cheb = np.polynomial.chebyshev.Chebyshev.fit(d, P, DEG, domain=[0.0, 1.0])
    coef = cheb.convert(kind=np.polynomial.Polynomial).coef
    coef = np.resize(coef, DEG + 1)
    coef[0] = 0.0
    return coef[1:].astype(np.float32)  # c_1 .. c_DEG


def _build_program(reps=1):
    nc = bass.Bass()
    rsj = nc.declare_dram_parameter("rsj", [3, 128, N], F32, isOutput=False)
    rsi = nc.declare_dram_parameter("rsi", [ROWS, 3], F32, isOutput=False)
    coefa = nc.declare_dram_parameter("coefa", [128, DEG], F32, isOutput=False)
    coefb = nc.declare_dram_parameter("coefb", [128, DEG], F32, isOutput=False)
    # Shape-bearing tag input: makes each reps-variant a distinct HLO module
    # (the NEFF compile cache keys on module fingerprint, which would
    # otherwise collide across reps since all real I/O shapes match).
    repstag = nc.declare_dram_parameter("repstag", [reps, 1], F32, isOutput=False)
    out = nc.declare_dram_parameter("out", [ROWS, 3], F32, isOutput=True)

    with TileContext(nc) as tc:
        with (
            tc.tile_pool(name="const", bufs=1) as cpool,
            tc.tile_pool(name="work", bufs=2) as wpool,
            tc.tile_pool(name="small", bufs=2) as spool,
        ):
            # Replicated j-coordinates, one [128, N] tile per coordinate.
            J = []
            for c in range(3):
                t = cpool.tile([128, N], F32, name=f"J{c}", tag=f"J{c}")
                nc.gpsimd.dma_start(out=t[:], in_=rsj[c])
                J.append(t)
            cA = cpool.tile([128, DEG], F32, tag="cA")
            nc.gpsimd.dma_start(out=cA[:], in_=coefa[:])
            cB = cpool.tile([128, DEG], F32, tag="cB")
            nc.gpsimd.dma_start(out=cB[:], in_=coefb[:])
            rtag = cpool.tile([1, 1], F32, tag="rtag")
            nc.gpsimd.dma_start(out=rtag[:], in_=repstag[reps - 1:reps, :])
            rsib = []
            for ib in range(NIB):
                t = cpool.tile([128, 3], F32, name=f"rsi{ib}", tag=f"rsi{ib}")
                nc.gpsimd.dma_start(out=t[:], in_=rsi[ib * 128:(ib + 1) * 128, :])
                rsib.append(t)

            for rep_ib in range(reps * NIB):
                ib = rep_ib % NIB
                sums = [spool.tile([128, NJT], F32, name=f"sums{c}", tag=f"sums{c}") for c in range(3)]
                for jt in range(NJT):
                    coef = cA if jt < NJT // 2 else cB
                    jsl = slice(jt * JT, (jt + 1) * JT)
                    # u = J - rs_i  in (-10, 10); minimum-image wrap via binary
                    # comparisons (no fp mod on this walrus; Sign() is unusable
                    # because Sign(0)=0 collapses |m| to 0 for pairs with
                    # u == +-5.0 exactly, which do occur among 50M pairs):
                    #   u1 = u - 10*(u >= 5);  m = u1 + 10*(u1 < -5)
                    # At |u|==5 exactly this yields |m|==5, where the force is
                    # exactly 0, so the (sign-ambiguous) boundary is harmless.
                    # Engine split (HW-measured): ACT instructions carry ~2us
                    # fixed cost each on this part, so ACT is reduced to the
                    # single mandatory Exp; everything else is DVE/GpSimd,
                    # balanced so neither engine dominates.
                    m = []
                    for c in range(3):
                        u = wpool.tile([128, JT], F32, name=f"u{c}", tag=f"u{c}")
                        nc.gpsimd.tensor_scalar(
                            u[:], J[c][:, jsl], rsib[ib][:, c:c + 1], None,
                            AOP.subtract)
                        ca = wpool.tile([128, JT], F32, name=f"ca{c}", tag=f"ca{c}")
                        nc.gpsimd.tensor_scalar(
                            ca[:], u[:], 5.0, 10.0, AOP.is_ge, AOP.mult)
                        E1 = nc.gpsimd if c < 2 else nc.vector
                        u1 = wpool.tile([128, JT], F32, name=f"u1{c}", tag=f"u1{c}")
                        E1.tensor_tensor(u1[:], u[:], ca[:], AOP.subtract)
                        cb = wpool.tile([128, JT], F32, name=f"cb{c}", tag=f"cb{c}")
                        nc.gpsimd.tensor_scalar(
                            cb[:], u1[:], -5.0, 10.0, AOP.is_lt, AOP.mult)
                        E2 = nc.gpsimd if c < 1 else nc.vector
                        mc = wpool.tile([128, JT], F32, name=f"m{c}", tag=f"m{c}")
                        E2.tensor_tensor(mc[:], u1[:], cb[:], AOP.add)
                        m.append(mc)
                    sq = []
                    for c in range(3):
                        s = wpool.tile([128, JT], F32, name=f"sq{c}", tag=f"sq{c}")
                        nc.vector.tensor_tensor(s[:], m[c][:], m[c][:], AOP.mult)
                        sq.append(s)
                    s3 = wpool.tile([128, JT], F32, tag="s3")
                    nc.vector.tensor_tensor(s3[:], sq[0][:], sq[1][:], AOP.add)
                    r2 = wpool.tile([128, JT], F32, tag="r2")
                    nc.vector.tensor_tensor(r2[:], s3[:], sq[2][:], AOP.add)
                    # g = clamp(1 - 0.04*r2, >= GMIN);  v = 1/g exactly on DVE
                    g = wpool.tile([128, JT], F32, tag="g")
                    nc.vector.tensor_scalar(
                        g[:], r2[:], -0.04, 1.0, AOP.mult, AOP.add)
                    gc = wpool.tile([128, JT], F32, tag="gc")
                    nc.vector.tensor_scalar(gc[:], g[:], GMIN, None, AOP.max)
                    v = wpool.tile([128, JT], F32, tag="v")
                    nc.vector.reciprocal(v[:], gc[:])
                    dcy = wpool.tile([128, JT], F32, tag="dcy")
                    nc.scalar.activation(dcy[:], v[:], AF.Exp, bias=1.0,
                                         scale=-1.0)
                    # Horner: F = (((c_D*d + c_{D-1})*d + ...)*d + c_1)*d
                    # via u_k = (u_{k+1} + c_k)*d, u_D = c_D*d; exact since c_0 = 0.
                    acc = wpool.tile([128, JT], F32, tag="acc0")
                    nc.vector.tensor_scalar(
                        acc[:], dcy[:], coef[:, DEG - 1:DEG], None, AOP.mult)
                    for k in range(DEG - 1, 0, -1):
                        nxt = wpool.tile([128, JT], F32, name=f"acc{(DEG - k) % 2}", tag=f"acc{(DEG - k) % 2}")
                        nc.vector.scalar_tensor_tensor(
                            nxt[:], acc[:], coef[:, k - 1:k], dcy[:],
                            AOP.add, AOP.mult)
                        acc = nxt
                    # Row-sums of F*m_c  (accumulated per j-tile into sums[c])
                    for c in range(3):
                        scratch = wpool.tile([128, JT], F32, tag="scratch")
                        nc.vector.scalar_tensor_tensor(
                            scratch[:], acc[:], 0.0, m[c][:],
                            AOP.bypass, AOP.mult,
                            accum_out=sums[c][:, jt:jt + 1])
                # Finalize block: out_rows = rs_i - sum(F*m)   (m = -true disp)
                res = spool.tile([128, 3], F32, tag="res")
                for c in range(3):
                    tot = spool.tile([128, 1], F32, name=f"tot{c}", tag=f"tot{c}")
                    nc.vector.tensor_reduce(
                        tot[:], sums[c][:], mybir.AxisListType.X, AOP.add)
                    nc.vector.tensor_scalar(
                        res[:, c:c + 1], tot[:], rsib[ib][:, c:c + 1], -1.0,
                        AOP.subtract, AOP.mult)
                nc.sync.dma_start(out=out[ib * 128:(ib + 1) * 128, :], in_=res[:])
    return nc


def _split_multi_waits(bir_json: bytes) -> bytes:
    """This walrus build rejects instructions carrying more than one sync
    wait ("Too many sync wait commands").  Hoist all-but-one wait of every
    instruction onto injected same-engine NoOps placed immediately before it
    (same blocking point on that engine's sequencer, so semantics are
    unchanged)."""
    import json as _json
    d = _json.loads(bir_json)
    for fn in d["functions"]:
        for blk in fn["blocks"]:
            new_insts = []
            for inst in blk["instructions"]:
                si = inst.get("sync_info")
                waits = (si or {}).get("on_wait") or []
                if len(waits) > 1:
                    for i, w in enumerate(waits[:-1]):
                        new_insts.append({
                            "debug": inst.get("debug", 0),
                            "engine": inst["engine"],
                            "ins": [],
                            "outs": [],
                            "name": f"{inst['name']}-w{i}",
                            "opcode": "NoOp",
                            "text_hint": "split_wait",
                            "sync_info": {"on_update": [], "on_wait": [w]},
                        })
                    si["on_wait"] = [waits[-1]]
                new_insts.append(inst)
            blk["instructions"] = new_insts
    return _json.dumps(d).encode()


def _get_program(reps=1):
    if reps not in _CACHED:
        nc = _build_program(reps)
        orig = nc.to_json_bytes
        nc.to_json_bytes = lambda: _split_multi_waits(orig())
        _CACHED[reps] = nc
    return _CACHED[reps]


def kernel(rs, same_w1, same_b1, same_wo, same_bo,
           diff_w1, diff_b1, diff_wo, diff_bo):
    global LAST_RESULTS
    rs = np.ascontiguousarray(np.asarray(rs, np.float32))
    coef_same = _fit_poly(same_w1, same_b1, same_wo, same_bo)
    coef_diff = _fit_poly(diff_w1, diff_b1, diff_wo, diff_bo)
    cs = np.ascontiguousarray(np.broadcast_to(coef_same[None, :], (128, DEG)))
    cd = np.ascontiguousarray(np.broadcast_to(coef_diff[None, :], (128, DEG)))

    rsj = np.ascontiguousarray(
        np.broadcast_to(rs.T[:, None, :], (3, 128, N)).astype(np.float32))

    in_maps = []
    for core in range(NCORES):
        up = (core * ROWS) < N_UP  # this core's rows are all one spin block
        in_maps.append({
            "rsj": rsj,
            "rsi": np.ascontiguousarray(rs[core * ROWS:(core + 1) * ROWS, :]),
            "coefa": cs if up else cd,   # coeffs for j < 2048
            "coefb": cd if up else cs,   # coeffs for j >= 2048
            "repstag": np.zeros((1, 1), np.float32),
        })

    nc = _get_program()
    LAST_RESULTS = run_bass_kernel_spmd(nc, in_maps, list(range(NCORES)))
    outs = [np.asarray(LAST_RESULTS.results[i]["out"]) for i in range(NCORES)]
    return np.concatenate(outs, axis=0).astype(np.float32)



# revision 2
# speedup vs baseline: 1.2362x; 1.2362x over previous
"""Trainium2 Bass kernel for the neural-backflow problem — Fourier method.

backflow_c[i] = sum_j H_c^{spin}(r_i - r_j), H_c(D) = wrap(D)_c * F(|wrap(D)|)
is a C-infinity function on the period-10 torus, so it factorizes through a
truncated 3D Fourier series (|k|_inf <= P = 12):

  sum_j H_c(r_i - r_j) = sum_q  M_c[q] * phi_q(r_i),
  M_c[q] = Wtil_c[q] * S[perm_c(q)],     S[q] = sum_j phi_q(r_j),

with phi_q the separable real trig product basis (per axis: cos(k*theta),
sin(k*theta), theta = omega*(v-5) in [-pi, pi)) and Wtil_c host-precomputed
amplitude tensors (FFT of H_c at kernel-call time; rs-independent).

Device pipeline per core (512 electrons' rows):
  1. features: one Sin activation (theta in range) + Chebyshev recurrence
  2. GXY[j, qx, qy] = phix*phiy per 128-j block (broadcast-AP mults)
  3. S[qz, qx*qy] via PE matmuls (f32r, full rate), separate up/dn spins
  4. M_c = WtilA*S_A + WtilB*S_B (crossed half-reads implement the C/S swap)
  5. back-transform: T1 = zfT.T @ M_c on PE; B = sum(T1 * GXY_i) fused
     multiply-reduce on DVE, seeded with rs_i so out = rs + backflow directly.

Self-pairs contribute exactly zero (every retained mode is odd in D_c).
"""

import numpy as np

import concourse.bass as bass
import concourse.mybir as mybir
from concourse.tile import TileContext
from concourse.bass_utils import run_bass_kernel_spmd

L = 10.0
N = 4096
N_UP = 2048
NCORES = 8
ROWS = N // NCORES          # 512 rows per core
NBLK = N // 128             # 32 j-blocks
NIB = ROWS // 128           # 4 i-blocks per core
P = 12                      # max mode per axis
Q = P + 1                   # 13: cos slots k=0..P (also sin slots k=0..P)
R = 2 * Q                   # 26 basis funcs per axis (S_0 slot identically 0)
RR = R * R                  # 676
HALF = Q * R                # 338 (qx half x full qy)
OMEGA = 2.0 * np.pi / L

F32 = mybir.dt.float32
F32R = mybir.dt.float32r
BF16 = mybir.dt.bfloat16
AOP = mybir.AluOpType
AF = mybir.ActivationFunctionType

LAST_RESULTS = None
_CACHED = {}

# ---------------------------------------------------------------- host side


def _neural_decayed_np(x, w1, b1, wo, bo):
    x_cut = L / 2
    xn = np.clip(x / x_cut, 0.0, 1.0 - 1e-05)
    decay = np.exp(1.0 - 1.0 / (1.0 - xn ** 2))
    z = decay[:, None] @ w1 + b1
    h = z / (1.0 + np.exp(-z))
    return ((h @ wo + bo).ravel()) * decay


def _amp_tensors(w1, b1, wo, bo, G=96):
    """A_c[kx,ky,kz] (k=0..P): amplitude of sin(k_c w D_c)*prod cos(k w D)
    in H_c(D) = wrap(D)_c * F(|wrap(D)|)."""
    w1 = np.asarray(w1, np.float64).reshape(1, -1)
    b1 = np.asarray(b1, np.float64).ravel()
    wo = np.asarray(wo, np.float64).reshape(-1, 1)
    bo = np.asarray(bo, np.float64).ravel()
    g = np.arange(G) * (L / G)
    X, Y, Z = np.meshgrid(g, g, g, indexing="ij")
    wrap = lambda u: (u + L / 2) % L - L / 2
    mx, my, mz = wrap(X), wrap(Y), wrap(Z)
    r = np.sqrt(mx ** 2 + my ** 2 + mz ** 2)
    F = _neural_decayed_np(r.ravel(), w1, b1, wo, bo).reshape(r.shape)
    ks = np.arange(Q)
    ang = np.outer(g, ks) * OMEGA
    Bc = np.cos(ang) * (2.0 / G)
    Bc[:, 0] *= 0.5
    Bs = np.sin(ang) * (2.0 / G)
    A = []
    for c, m in enumerate((mx, my, mz)):
        H = m * F
        mats = [Bc, Bc, Bc]
        mats[c] = Bs
        t = np.einsum("abc,ax->xbc", H, mats[0], optimize=True)
        t = np.einsum("xbc,by->xyc", t, mats[1], optimize=True)
        t = np.einsum("xyc,cz->xyz", t, mats[2], optimize=True)
        A.append(t)
    return A


def _w_dev(A):
    """Device amplitude tensors W_dev[c][qz, qx, qy] (f32), including the
    sin-slot sign on the swap axis and the half-swap permutation baked in
    for c=0 (qx) and c=1 (qy).  Basis slots: q<Q: cos(k), q>=Q: sin(k=q-Q)."""
    kq = np.concatenate([np.arange(Q), np.arange(Q)])      # k per slot
    sin_slot = np.arange(R) >= Q
    out = []
    for c in range(3):
        Wc = A[c][np.ix_(kq, kq, kq)]                      # [qx,qy,qz]
        sgn = np.where(sin_slot, -1.0, 1.0)
        if c == 0:
            Wc = Wc * sgn[:, None, None]
        elif c == 1:
            Wc = Wc * sgn[None, :, None]
        else:
            Wc = Wc * sgn[None, None, :]
        # zero the identically-zero sin(k=0) slots for cleanliness
        z = np.zeros(R, bool)
        z[Q] = True
        Wc[z, :, :] = 0.0
        Wc[:, z, :] = 0.0
        Wc[:, :, z] = 0.0
        Wc = np.transpose(Wc, (2, 0, 1))                   # [qz, qx, qy]
        perm = np.concatenate([np.arange(Q, R), np.arange(Q)])
        if c == 0:
            Wc = Wc[:, perm, :]
        elif c == 1:
            Wc = Wc[:, :, perm]
        out.append(Wc.astype(np.float32))
    return out


def host_prepare(rs, same_w1, same_b1, same_wo, same_bo,
                 diff_w1, diff_b1, diff_wo, diff_bo):
    """Build per-core input maps (without repstag)."""
    rs = np.ascontiguousarray(np.asarray(rs, np.float32))
    A_same = _amp_tensors(same_w1, same_b1, same_wo, same_bo)
    A_diff = _amp_tensors(diff_w1, diff_b1, diff_wo, diff_bo)
    W_same = _w_dev(A_same)    # list of 3 [R, R, R] = [qz, qx, qy]
    W_diff = _w_dev(A_diff)
    wS = np.ascontiguousarray(np.stack(W_same, axis=1).reshape(R, 3, R, R))
    wD = np.ascontiguousarray(np.stack(W_diff, axis=1).reshape(R, 3, R, R))
    rsP = np.ascontiguousarray(
        rs.reshape(NBLK, 128, 3).transpose(1, 0, 2))       # [128, 32, 3]
    idn = np.ascontiguousarray(np.eye(128, dtype=np.float32))
    in_maps = []
    for core in range(NCORES):
        up = (core * ROWS) < N_UP
        own = list(range(core * NIB, (core + 1) * NIB))
        same = [b for b in (range(0, 16) if up else range(16, 32))
                if b not in own]
        other = list(range(16, 32) if up else range(0, 16))
        order = own + same + other
        # device blocks 0..3 = this core's rows; 0..15 = same-spin electrons
        in_maps.append({
            "rsP": np.ascontiguousarray(rsP[:, order, :]),
            "wA": wS,                 # S over device blocks 0..15: same spin
            "wB": wD,                 # S over device blocks 16..31: other
            "idn": idn,
        })
    return in_maps


# -------------------------------------------------------------- device side


def _build_program(reps=1):
    nc = bass.Bass()
    rsP = nc.declare_dram_parameter("rsP", [128, NBLK, 3], F32, isOutput=False)
    wA = nc.declare_dram_parameter("wA", [R, 3, R, R], F32, isOutput=False)
    wB = nc.declare_dram_parameter("wB", [R, 3, R, R], F32, isOutput=False)
    idn = nc.declare_dram_parameter("idn", [128, 128], F32, isOutput=False)
    repstag = nc.declare_dram_parameter("repstag", [reps, 1], F32, isOutput=False)
    outp = nc.declare_dram_parameter("out", [ROWS, 3], F32, isOutput=True)

    # SPMD: the host reorders rsP's blocks per core so device blocks 0..3 are
    # this core's own rows and 0..15 are same-spin electrons (see host_prepare)

    with TileContext(nc) as tc:
        with (
            tc.tile_pool(name="const", bufs=1) as cpool,
            tc.tile_pool(name="feat", bufs=1) as fpool,
            tc.tile_pool(name="work", bufs=2) as wpool,
            tc.tile_pool(name="small", bufs=2) as spool,
            tc.tile_pool(name="psum", bufs=1, space="PSUM") as ppool,
            tc.tile_pool(name="psum2", bufs=2, space="PSUM") as ppool2,
        ):
            RSP = cpool.tile([128, NBLK, 3], F32, tag="RSP")
            nc.gpsimd.dma_start(out=RSP[:], in_=rsP[:, :, :])
            WA = cpool.tile([R, 3, R, R], F32, tag="WA")
            nc.gpsimd.dma_start(out=WA[:], in_=wA[:, :, :, :])
            WB = cpool.tile([R, 3, R, R], F32, tag="WB")
            nc.gpsimd.dma_start(out=WB[:], in_=wB[:, :, :, :])
            IDN = cpool.tile([128, 128], F32, tag="IDN")
            nc.gpsimd.dma_start(out=IDN[:], in_=idn[:, :])
            IDNb = cpool.tile([128, 128], BF16, tag="IDNb")
            nc.gpsimd.tensor_copy(IDNb[:], IDN[:])
            rtag = cpool.tile([1, 1], F32, tag="rtag")
            nc.gpsimd.dma_start(out=rtag[:], in_=repstag[reps - 1:reps, :])

            for rep in range(reps):
                # ---------------- stage 0: features --------------------
                # FE[p, b, c, q]: q<Q: cos(k theta), q>=Q: sin(k theta)
                FE = fpool.tile([128, NBLK, 3, R], F32, tag="FE")
                TH = spool.tile([128, NBLK, 3, 1], F32, tag="TH")
                nc.gpsimd.tensor_scalar(
                    TH[:], RSP[:].unsqueeze(3), float(OMEGA),
                    float(-5.0 * OMEGA), AOP.mult, AOP.add)
                TH2 = spool.tile([128, NBLK, 3, 1], F32, tag="TH2")
                nc.gpsimd.tensor_scalar(TH2[:], TH[:], 0.5, None, AOP.mult)
                # C_0 = 1, S_0 = 0
                nc.gpsimd.memset(FE[:, :, :, 0:1], 1.0)
                nc.gpsimd.memset(FE[:, :, :, Q:Q + 1], 0.0)
                # S_1 = sin(theta); SH = sin(theta/2); C_1 = 1 - 2 SH^2
                nc.scalar.activation(FE[:, :, :, Q + 1:Q + 2], TH[:], AF.Sin)
                SH = spool.tile([128, NBLK, 3, 1], F32, tag="SH")
                nc.scalar.activation(SH[:], TH2[:], AF.Sin)
                SQ = spool.tile([128, NBLK, 3, 1], F32, tag="SQ")
                nc.vector.tensor_tensor(SQ[:], SH[:], SH[:], AOP.mult)
                nc.vector.tensor_scalar(
                    FE[:, :, :, 1:2], SQ[:], -2.0, 1.0, AOP.mult, AOP.add)
                TW = spool.tile([128, NBLK, 3, 1], F32, tag="TW")
                nc.vector.tensor_scalar(
                    TW[:], FE[:, :, :, 1:2], 2.0, None, AOP.mult)
                # Chebyshev recurrence, split blocks across DVE / GpSimd
                HB = NBLK // 2
                halves = [(nc.vector, slice(0, HB)), (nc.gpsimd, slice(HB, NBLK))]
                for k in range(2, Q):
                    for eng, bs in halves:
                        # V_k = 2c1 * V_{k-1} - V_{k-2}  for both C and S rows
                        t = wpool.tile([128, HB, 3, 2], F32,
                                       name=f"rec{bs.start}", tag=f"rec{bs.start}")
                        v1 = FE[:, bs, :, k - 1::Q]   # cols {k-1, Q+k-1}
                        v2 = FE[:, bs, :, k - 2::Q]
                        eng.tensor_tensor(
                            t[:], TW[:, bs, :, :].broadcast_to([128, HB, 3, 2]),
                            v1, AOP.mult)
                        eng.tensor_tensor(FE[:, bs, :, k::Q], t[:], v2,
                                          AOP.subtract)

                # bf16 copy of z-features (matmul operands must be bf16)
                FEZb = fpool.tile([128, NBLK, R], BF16, tag="FEZb")
                nc.vector.tensor_copy(FEZb[:], FE[:, :, 2, :])

                # ---------------- stage 0b: zfT transposes ------------
                zfT = []
                zfTx = []
                for ib in range(NIB):
                    tp = ppool2.tile([R, 128], BF16, name=f"tp{ib}", tag="tp")
                    nc.tensor.matmul(tp[:], FEZb[:, ib, :], IDNb[:],
                                     is_transpose=True)
                    zn = fpool.tile([R, 128], BF16, name=f"zfT{ib}", tag=f"zfT{ib}")
                    nc.scalar.copy(zn[:], tp[:])
                    zfT.append(zn)
    # z-features with C/S halves swapped, then transposed
                    fzx = wpool.tile([128, R], BF16, name=f"fzx{ib}", tag="fzx")
                    nc.gpsimd.tensor_copy(fzx[:, 0:Q], FEZb[:, ib, Q:R])
                    nc.gpsimd.tensor_copy(fzx[:, Q:R], FEZb[:, ib, 0:Q])
                    tpx = ppool2.tile([R, 128], BF16, name=f"tpx{ib}", tag="tp")
                    nc.tensor.matmul(tpx[:], fzx[:], IDNb[:], is_transpose=True)
                    zx = fpool.tile([R, 128], BF16, name=f"zfTx{ib}", tag=f"zfTx{ib}")
                    nc.scalar.copy(zx[:], tpx[:])
                    zfTx.append(zx)

                # ---------------- stage 1+2: GXY + S ------------------
                SPS = [[ppool.tile([R, Q, R], F32, name=f"S{s}{h}", tag=f"S{s}{h}")
                        for h in range(2)] for s in range(2)]
                GI = []
                DVE_BLOCKS = 18
                for b in range(NBLK):
                    s = 0 if b < NBLK // 2 else 1
                    if b < NIB:
                        g = fpool.tile([128, R, R], BF16, name=f"GI{b}",
                                       tag=f"GI{b}")
                        GI.append(g)
                    else:
                        g = wpool.tile([128, R, R], BF16, name="gxy", tag="gxy")
                    eng = nc.vector if (b % NBLK) < DVE_BLOCKS else nc.gpsimd
                    eng.tensor_tensor(
                        g[:],
                        FE[:, b, 0, :].unsqueeze(2).broadcast_to([128, R, R]),
                        FE[:, b, 1, :].unsqueeze(1).broadcast_to([128, R, R]),
                        AOP.mult)
                    first = b % (NBLK // 2) == 0
                    last = b % (NBLK // 2) == NBLK // 2 - 1
                    for h in range(2):
                        nc.tensor.matmul(
                            SPS[s][h][:],
                            FEZb[:, b, :],
                            g[:, h * Q:(h + 1) * Q, :],
                            start=first, stop=last)

                # S psum -> sbuf (ACT copies)
                SS = []
                for s in range(2):
                    t = spool.tile([R, 2, Q, R], F32, name=f"SS{s}", tag=f"SS{s}")
                    for h in range(2):
                        nc.scalar.copy(t[:, h], SPS[s][h][:])
                    SS.append(t)

                # ---------------- stage 3: M_c ------------------------
                # SS layout [qz, (2,Q)=qx, qy]; W layout [qz, c, qx, qy]
                M = []
                for c in range(3):
                    m = spool.tile([R, 2, Q, R], BF16, name=f"M{c}", tag=f"M{c}")
                    t1 = wpool.tile([R, 2, Q, R], F32, name="mt1", tag="mt1")
                    t2 = wpool.tile([R, 2, Q, R], F32, name="mt2", tag="mt2")
                    WAc = WA[:, c].rearrange("z (hx q) y -> z hx q y", hx=2)
                    WBc = WB[:, c].rearrange("z (hx q) y -> z hx q y", hx=2)
                    eng = nc.vector if c != 1 else nc.gpsimd
                    if c == 0:
                        # crossed half read on qx
                        for h in range(2):
                            eng.tensor_tensor(t1[:, h], WAc[:, h], SS[0][:, 1 - h],
                                              AOP.mult)
                            eng.tensor_tensor(t2[:, h], WBc[:, h], SS[1][:, 1 - h],
                                              AOP.mult)
                            eng.tensor_tensor(m[:, h], t1[:, h], t2[:, h], AOP.add)
                    elif c == 1:
                        # crossed read on qy inner (2, Q) split
                        WAc5 = WA[:, c].rearrange(
                            "z x (hy q) -> z x hy q", hy=2)
                        WBc5 = WB[:, c].rearrange(
                            "z x (hy q) -> z x hy q", hy=2)
                        SS05 = [SS[0].rearrange("z hx q (hy p) -> z (hx q) hy p", hy=2),
                                SS[1].rearrange("z hx q (hy p) -> z (hx q) hy p", hy=2)]
                        m5 = m.rearrange("z hx q (hy p) -> z (hx q) hy p", hy=2)
                        t15 = t1.rearrange("z hx q (hy p) -> z (hx q) hy p", hy=2)
                        t25 = t2.rearrange("z hx q (hy p) -> z (hx q) hy p", hy=2)
                        for s in range(2):
                            eng.tensor_tensor(t15[:, :, s], WAc5[:, :, s],
                                              SS05[0][:, :, 1 - s], AOP.mult)
                            eng.tensor_tensor(t25[:, :, s], WBc5[:, :, s],
                                              SS05[1][:, :, 1 - s], AOP.mult)
                        eng.tensor_tensor(m5[:], t15[:], t25[:], AOP.add)
                    else:
                        eng.tensor_tensor(t1[:], WAc, SS[0][:], AOP.mult)
                        eng.tensor_tensor(t2[:], WBc, SS[1][:], AOP.mult)
                        eng.tensor_tensor(m[:], t1[:], t2[:], AOP.add)
                    M.append(m)

                # ---------------- stage 4: back-transform -------------
                for ib in range(NIB):
                    res = spool.tile([128, 3], F32, name=f"res{ib}", tag="res")
                    gi2 = GI[ib][:].rearrange("p a b -> p (a b)")
                    for c in range(3):
                        lhsT = (zfT[ib] if c != 2 else zfTx[ib])
                        accs = []
                        for h in range(2):
                            t1p = ppool2.tile([128, HALF], F32,
                                              name=f"T1_{ib}{c}{h}", tag="T1")
                            nc.tensor.matmul(
                                t1p[:], lhsT[:], M[c][:, h],
                                start=True, stop=True)
                            scr = wpool.tile([128, HALF], F32, name="scr",
                                             tag="scr")
                            acc = spool.tile([128, 1], F32,
                                             name=f"acc{ib}{c}{h}",
                                             tag=f"acc{h}")
                            nc.vector.scalar_tensor_tensor(
                                scr[:], t1p[:], 0.0,
                                gi2[:, h * HALF:(h + 1) * HALF],
                                AOP.bypass, AOP.mult, accum_out=acc[:])
                            accs.append(acc)
                        # res_c = (acc0 + rs_i_c) + acc1
                        nc.vector.scalar_tensor_tensor(
                            res[:, c:c + 1], accs[0][:], RSP[:, ib, c:c + 1],
                            accs[1][:], AOP.add, AOP.add)
                    nc.sync.dma_start(
                        out=outp[ib * 128:(ib + 1) * 128, :], in_=res[:])
    return nc


def _split_multi_waits(bir_json: bytes) -> bytes:
    """Walrus rejects >1 sync wait per instruction; hoist extras onto
    same-engine NoOps immediately before (same blocking semantics)."""
    import json as _json
    d = _json.loads(bir_json)
    for fn in d["functions"]:
        for blk in fn["blocks"]:
            new_insts = []
            for inst in blk["instructions"]:
                si = inst.get("sync_info")
                waits = (si or {}).get("on_wait") or []
                if len(waits) > 1:
                    for i, w in enumerate(waits[:-1]):
                        new_insts.append({
                            "debug": inst.get("debug", 0),
                            "engine": inst["engine"],
                            "ins": [], "outs": [],
                            "name": f"{inst['name']}-w{i}",
                            "opcode": "NoOp",
                            "text_hint": "split_wait",
                            "sync_info": {"on_update": [], "on_wait": [w]},
                        })
                    si["on_wait"] = [waits[-1]]
                new_insts.append(inst)
            blk["instructions"] = new_insts
    return _json.dumps(d).encode()


def _get_program(reps=1):
    if reps not in _CACHED:
        nc = _build_program(reps)
        orig = nc.to_json_bytes
        nc.to_json_bytes = lambda: _split_multi_waits(orig())
        _CACHED[reps] = nc
    return _CACHED[reps]


def kernel(rs, same_w1, same_b1, same_wo, same_bo,
           diff_w1, diff_b1, diff_wo, diff_bo):
    global LAST_RESULTS
    in_maps = host_prepare(rs, same_w1, same_b1, same_wo, same_bo,
                           diff_w1, diff_b1, diff_wo, diff_bo)
    for im in in_maps:
        im["repstag"] = np.zeros((1, 1), np.float32)
    nc = _get_program()
    LAST_RESULTS = run_bass_kernel_spmd(nc, in_maps, list(range(NCORES)))
    outs = [np.asarray(LAST_RESULTS.results[i]["out"]) for i in range(NCORES)]
    return np.concatenate(outs, axis=0).astype(np.float32)
